# revision 16
# baseline (speedup 1.0000x reference)
"""BOW classifier kernel for 8 Trainium2 NeuronCores.

Vocab-sharded counts-matmul formulation.  The masked mean-pool
  pooled[b] = (1/len[b]) * sum_{s<len[b]} emb[text[s,b]]
is a sparse matmul  pooled = counts @ emb  with counts[b,v] the number of
times token v appears in the first len[b] positions of column b (the
1/len is folded into counts on the host).  Each core owns a 6272-row
slice of the (padded, bf16) embedding table and the matching slice of
counts^T, computes its partial pooled on the tensor engine (bf16 x bf16
-> fp32 PSUM), and a bf16 ReduceScatter sums the partials and hands core
i batch rows [128*i, 128*(i+1)).  The MLP tail (fc1 bias fold + relu +
fc2, bf16 inputs with fp32 PSUM accumulate) runs per-core on its 128
batch rows.

Schedule notes: dummy matmuls on memset tiles ramp the PE pstate during
the initial DMA fill (the real accumulation opens with start=True, so
the junk is discarded); counts DMAs issue two 128-row chunks per
instruction from the sync engine while embedding DMAs go through the
scalar-engine HWDGE and weight DMAs through gpsimd, giving three
parallel issue paths so the PE (~1.0 us/chunk consume rate) never
starves.  The ReduceScatter is triggered as soon as the accumulator
drains land; its start is pinned by NRT's fixed first-collective
barrier (~70 us), which the matmul phase hides.
"""

import sys

import numpy as np

for _p in ("/opt/trn_rl_repo",):
    if _p not in sys.path:
        sys.path.insert(0, _p)

V, E, H, O = 50000, 300, 512, 2
S, B = 512, 1024
NCORES = 8
VSH = 6272          # padded vocab rows per core (49 * 128)
VP = NCORES * VSH   # 50176 padded vocab rows total
KC = VSH // 128     # 49 contraction chunks per core
BG = B // 128       # 8 batch groups of 128
BS = B // NCORES    # 128 batch rows per core after reduce-scatter
NWARM = 30          # dummy matmuls to ramp the PE pstate


def _build_nc(repeat=None):
    import os
    from contextlib import ExitStack

    if repeat is None:
        repeat = int(os.environ.get("KERNEL_REPEAT", "1"))

    import concourse.tile as tile
    from concourse import bacc, bass, mybir
    from concourse.masks import make_identity

    bf16, f32 = mybir.dt.bfloat16, mybir.dt.float32

    nc = bacc.Bacc(None, target_bir_lowering=False, num_devices=NCORES)
    cnt_d = nc.declare_dram_parameter("cnt", [VSH, B], bf16, isOutput=False)
    emb_d = nc.declare_dram_parameter("emb", [VSH, E], bf16, isOutput=False)
    w1b_d = nc.declare_dram_parameter("w1b", [E + 1, H], bf16, isOutput=False)
    w2b_d = nc.declare_dram_parameter("w2b", [H + 1, O], bf16, isOutput=False)
    out_d = nc.declare_dram_parameter("out", [BS, O], f32, isOutput=True)

    with tile.TileContext(nc) as tc, ExitStack() as ctx:
        sb = ctx.enter_context(tc.tile_pool(name="sb", bufs=1))
        dram = ctx.enter_context(tc.tile_pool(name="dram", bufs=1, space="DRAM"))

        # counts: two 128-row chunks per DMA, alternating between the two
        # HWDGE queues (sync and scalar); embeddings and weights on gpsimd
        cnt_t = []
        for j in range((KC + 1) // 2):
            r1 = min((j + 1) * 256, VSH)
            t2 = (r1 - j * 256) // 128
            ct = sb.tile([128, t2 * B], bf16, tag=f"cnt{j}", name=f"cnt{j}")
            eng = nc.sync if j % 2 == 0 else nc.scalar
            eng.dma_start(
                out=ct[:].rearrange("p (t c) -> p t c", t=t2),
                in_=cnt_d[j * 256:r1, :].rearrange("(t p) c -> p t c", t=t2),
            )
            cnt_t.append(ct)
        emb_t = []
        for k in range(KC):
            et = sb.tile([128, E], bf16, tag=f"emb{k}", name=f"emb{k}")
            nc.gpsimd.dma_start(out=et[:], in_=emb_d[k * 128:(k + 1) * 128, :])
            emb_t.append(et)

        w1_t = []
        for c, (r0, r1) in enumerate([(0, 128), (128, 256), (256, E + 1)]):
            t = sb.tile([r1 - r0, H], bf16, tag=f"w1_{c}", name=f"w1_{c}")
            nc.gpsimd.dma_start(out=t[:], in_=w1b_d[r0:r1, :])
            w1_t.append(t)
        w2_t = []
        for c in range(4):
            t = sb.tile([128, O], bf16, tag=f"w2_{c}", name=f"w2_{c}")
            nc.gpsimd.dma_start(out=t[:], in_=w2b_d[c * 128:(c + 1) * 128, :])
            w2_t.append(t)
        b2_t = sb.tile([1, O], bf16, tag="b2")
        nc.gpsimd.dma_start(out=b2_t[:], in_=w2b_d[H:H + 1, :])

        # PE pstate warm-up on memset tiles (no DMA dependency); the real
        # accumulation below opens with start=True, discarding this junk
        wa = sb.tile([128, 128], bf16, tag="wa")
        nc.vector.memset(wa[:], 0.0)
        wb = sb.tile([128, E], bf16, tag="wb")
        nc.vector.memset(wb[:], 0.0)

        pooled_all = sb.tile([128, BG * E], bf16, tag="pooled_all")
        with tc.tile_pool(name="psA", bufs=1, space="PSUM") as psA:
            acc = [
                psA.tile([128, 512], f32, tag=f"acc{g}", name=f"acc{g}")
                for g in range(BG)
            ]
            for w in range(NWARM):
                nc.tensor.matmul(out=acc[0][:, 0:E], lhsT=wa[:], rhs=wb[:],
                                 start=True, stop=True)
            for rep in range(repeat):
                for k in range(KC):
                    ct = cnt_t[k // 2]
                    t = k % 2
                    for g in range(BG):
                        nc.tensor.matmul(
                            out=acc[g][:, 0:E],
                            lhsT=ct[:, t * B + g * 128:t * B + (g + 1) * 128],
                            rhs=emb_t[k][:],
                            start=(k == 0),
                            stop=(k == KC - 1),
                        )
            # drain the accumulators (pipelines behind the last matmuls;
            # gpsimd cannot read PSUM)
            for g in range(BG):
                nc.vector.tensor_copy(
                    out=pooled_all[:, g * E:(g + 1) * E], in_=acc[g][:, 0:E]
                )

        # cross-core sum + scatter: core i keeps batch rows [128i, 128i+128)
        part_d = dram.tile([B, E], bf16)
        rs_d = dram.tile([BS, E], bf16)
        nc.gpsimd.dma_start(
            out=part_d[:].rearrange("(g p) e -> p g e", g=BG),
            in_=pooled_all[:].rearrange("p (g e) -> p g e", g=BG),
        )
        nc.gpsimd.collective_compute(
            "ReduceScatter",
            mybir.AluOpType.add,
            replica_groups=[list(range(NCORES))],
            ins=[part_d.opt()],
            outs=[rs_d.opt()],
        )
        pooled = sb.tile([BS, E], bf16, tag="pooled")
        nc.gpsimd.dma_start(out=pooled[:], in_=rs_d[:])

        with tc.tile_pool(name="ps", bufs=1, space="PSUM") as ps, \
                tc.tile_pool(name="ps2", bufs=2, space="PSUM") as ps2:
            # fc1: h = relu(pooled @ W1 + b1), contraction via pooled^T on PE
            ident = sb.tile([128, 128], bf16, tag="ident")
            make_identity(nc, ident[:])
            hp = ps.tile([128, H], f32, tag="hp", space="PSUM")
            # keep the PE pstate up through the collective (junk results;
            # the real fc1 accumulation opens with start=True)
            for w in range(48):
                nc.tensor.matmul(out=hp[:, 0:64], lhsT=wa[:],
                                 rhs=wb[:, 0:64], start=True, stop=True)
            lhs = []
            for c, (c0, c1) in enumerate([(0, 128), (128, 256), (256, E)]):
                w = c1 - c0
                pt = ps2.tile([w, 128], bf16, tag="tr", space="PSUM")
                nc.tensor.transpose(out=pt[:], in_=pooled[:, c0:c1],
                                    identity=ident[:])
                rows = w + 1 if c == 2 else w
                lt = sb.tile([rows, 128], bf16, tag=f"lhs{c}", name=f"lhs{c}")
                if c == 2:
                    # row `w` must be ones (bias row); memset whole tile first
                    # (partition-offset writes must start at partition 0)
                    nc.vector.memset(lt[:], 1.0)
                nc.vector.tensor_copy(out=lt[0:w, :], in_=pt[:])
                lhs.append(lt)
            for c in range(3):
                nc.tensor.matmul(
                    out=hp[:], lhsT=lhs[c][:], rhs=w1_t[c][:],
                    start=(c == 0), stop=(c == 2),
                )
            h = sb.tile([128, H], bf16, tag="h")
            nc.scalar.activation(out=h[:], in_=hp[:],
                                 func=mybir.ActivationFunctionType.Relu)

            # fc2: out = h @ W2 + b2
            ones1 = sb.tile([1, 128], bf16, tag="ones1")
            nc.vector.memset(ones1[:], 1.0)
            op_ = ps.tile([128, O], f32, tag="op", space="PSUM")
            for c in range(4):
                pt = ps2.tile([128, 128], bf16, tag="tr2", space="PSUM")
                nc.tensor.transpose(out=pt[:], in_=h[:, c * 128:(c + 1) * 128],
                                    identity=ident[:])
                ht = sb.tile([128, 128], bf16, tag=f"ht{c}", name=f"ht{c}")
                nc.vector.tensor_copy(out=ht[:], in_=pt[:])
                nc.tensor.matmul(out=op_[:], lhsT=ht[:], rhs=w2_t[c][:],
                                 start=(c == 0), stop=False)
            nc.tensor.matmul(out=op_[:], lhsT=ones1[:], rhs=b2_t[:],
                             start=False, stop=True)
            out_sb = sb.tile([BS, O], f32, tag="osb")
            nc.vector.tensor_copy(out=out_sb[:], in_=op_[:])
            # gpsimd queue: its last DMA completed recently, so the
            # completion semaphore lands fast (the idle sync queue takes
            # ~7 us to report completion of a late one-off DMA)
            nc.gpsimd.dma_start(out=out_d[:], in_=out_sb[:])

    nc.finalize()
    return nc


def _prep_in_maps(text, lengths, emb_table, W1, b1, W2, b2):
    import ml_dtypes

    bf16 = ml_dtypes.bfloat16
    text = np.asarray(text, dtype=np.int64)         # [S, B]
    lengths = np.asarray(lengths, dtype=np.int64)   # [B]

    # counts^T [VP, B] scaled by 1/len: row v = per-batch frequency of
    # token v among the first len[b] positions (vocab-major for sharding)
    mask = np.arange(S)[:, None] < lengths[None, :]
    flat = (text * B + np.arange(B)[None, :])[mask]
    cntT = np.bincount(flat, minlength=VP * B).reshape(VP, B)
    inv_len = (1.0 / lengths.astype(np.float32)).astype(np.float32)
    cntT16 = (cntT * inv_len[None, :]).astype(bf16)

    embp = np.zeros((VP, E), np.float32)
    embp[:V] = np.asarray(emb_table, np.float32)
    emb16 = embp.astype(bf16)

    w1b = np.vstack([np.asarray(W1, np.float32),
                     np.asarray(b1, np.float32)[None, :]]).astype(bf16)
    w2b = np.vstack([np.asarray(W2, np.float32),
                     np.asarray(b2, np.float32)[None, :]]).astype(bf16)

    in_maps = []
    for i in range(NCORES):
        in_maps.append({
            "cnt": np.ascontiguousarray(cntT16[i * VSH:(i + 1) * VSH]),
            "emb": np.ascontiguousarray(emb16[i * VSH:(i + 1) * VSH]),
            "w1b": w1b,
            "w2b": w2b,
        })
    return in_maps


def _run(inputs, trace=False):
    from concourse.bass_utils import run_bass_kernel_spmd

    nc = _build_nc()
    in_maps = _prep_in_maps(**inputs)
    res = run_bass_kernel_spmd(nc, in_maps, list(range(NCORES)), trace=trace)
    out = np.concatenate([res.results[i]["out"] for i in range(NCORES)], axis=0)
    return out.astype(np.float32), res


def kernel(**inputs):
    out, _ = _run(inputs, trace=False)
    return out


# revision 17
# speedup vs baseline: 1.0900x; 1.0900x over previous
"""BOW classifier kernel for 8 Trainium2 NeuronCores.

Vocab-sharded counts-matmul formulation.  The masked mean-pool
  pooled[b] = (1/len[b]) * sum_{s<len[b]} emb[text[s,b]]
is a sparse matmul  pooled = counts @ emb  with counts[b,v] the number of
times token v appears in the first len[b] positions of column b (the
1/len is folded into counts on the host).  Each core owns a 6272-row
slice of the (padded, bf16) embedding table and the matching slice of
counts^T, computes its partial pooled on the tensor engine (bf16 x bf16
-> fp32 PSUM), and a bf16 ReduceScatter sums the partials and hands core
i batch rows [128*i, 128*(i+1)).  The MLP tail runs per-core on its 128
batch rows: pooled^T lands via XBAR DMA-transpose straight out of the
collective buffer, fc1 computes h^T = relu(W1^T pooled^T + b1) so fc2
(out = h @ W2 + b2) needs no transposes at all; bf16 inputs, fp32 PSUM.

Schedule notes: dummy matmuls on memset tiles ramp the PE pstate during
the initial DMA fill (the real accumulation opens with start=True, so
the junk is discarded); count/embedding DMAs interleave across the two
HWDGE queues (sync + scalar; counts two 128-row chunks per instruction)
while gpsimd carries only the small transfers, keeping every issue path
ahead of the PE's ~1.0 us/chunk consume rate.  The ReduceScatter
triggers as soon as the accumulator drains land; its start is pinned by
NRT's fixed first-collective barrier (~70 us), which the matmul phase
hides.
"""

import sys

import numpy as np

for _p in ("/opt/trn_rl_repo",):
    if _p not in sys.path:
        sys.path.insert(0, _p)

V, E, H, O = 50000, 300, 512, 2
S, B = 512, 1024
NCORES = 8
VSH = 6272          # padded vocab rows per core (49 * 128)
VP = NCORES * VSH   # 50176 padded vocab rows total
KC = VSH // 128     # 49 contraction chunks per core
BG = B // 128       # 8 batch groups of 128
BS = B // NCORES    # 128 batch rows per core after reduce-scatter
NWARM = 30          # dummy matmuls to ramp the PE pstate


def _build_nc(repeat=None):
    import os
    from contextlib import ExitStack

    if repeat is None:
        repeat = int(os.environ.get("KERNEL_REPEAT", "1"))

    import concourse.tile as tile
    from concourse import bacc, bass, mybir
    from concourse.masks import make_identity

    bf16, f32 = mybir.dt.bfloat16, mybir.dt.float32

    nc = bacc.Bacc(None, target_bir_lowering=False, num_devices=NCORES)
    cnt_d = nc.declare_dram_parameter("cnt", [VSH, B], bf16, isOutput=False)
    emb_d = nc.declare_dram_parameter("emb", [VSH, E], bf16, isOutput=False)
    w1b_d = nc.declare_dram_parameter("w1b", [E + 1, H], bf16, isOutput=False)
    w2b_d = nc.declare_dram_parameter("w2b", [H + 1, O], bf16, isOutput=False)
    out_d = nc.declare_dram_parameter("out", [BS, O], f32, isOutput=True)

    with tile.TileContext(nc) as tc, ExitStack() as ctx:
        sb = ctx.enter_context(tc.tile_pool(name="sb", bufs=1))
        dram = ctx.enter_context(tc.tile_pool(name="dram", bufs=1, space="DRAM"))

        # counts (two 128-row chunks per instruction) and embeddings,
        # interleaved in chunk order across the two HWDGE queues
        cnt_t, emb_t = [], []
        for j in range((KC + 1) // 2):
            r1 = min((j + 1) * 256, VSH)
            t2 = (r1 - j * 256) // 128
            ct = sb.tile([128, t2 * B], bf16, tag=f"cnt{j}", name=f"cnt{j}")
            eng_c = nc.sync if j % 2 == 0 else nc.scalar
            eng_e = nc.scalar if j % 2 == 0 else nc.sync
            eng_c.dma_start(
                out=ct[:].rearrange("p (t c) -> p t c", t=t2),
                in_=cnt_d[j * 256:r1, :].rearrange("(t p) c -> p t c", t=t2),
            )
            cnt_t.append(ct)
            for k in range(2 * j, 2 * j + t2):
                et = sb.tile([128, E], bf16, tag=f"emb{k}", name=f"emb{k}")
                eng_e.dma_start(out=et[:],
                                in_=emb_d[k * 128:(k + 1) * 128, :])
                emb_t.append(et)

        w1_t = []
        for c, (r0, r1) in enumerate([(0, 128), (128, 256), (256, E + 1)]):
            t = sb.tile([r1 - r0, H], bf16, tag=f"w1_{c}", name=f"w1_{c}")
            nc.gpsimd.dma_start(out=t[:], in_=w1b_d[r0:r1, :])
            w1_t.append(t)
        w2_t = []
        for c in range(4):
            t = sb.tile([128, O], bf16, tag=f"w2_{c}", name=f"w2_{c}")
            nc.gpsimd.dma_start(out=t[:], in_=w2b_d[c * 128:(c + 1) * 128, :])
            w2_t.append(t)
        b2_t = sb.tile([1, O], bf16, tag="b2")
        nc.gpsimd.dma_start(out=b2_t[:], in_=w2b_d[H:H + 1, :])

        # PE pstate warm-up on memset tiles (no DMA dependency); the real
        # accumulation below opens with start=True, discarding this junk
        wa = sb.tile([128, 128], bf16, tag="wa")
        nc.vector.memset(wa[:], 0.0)
        wb = sb.tile([128, E], bf16, tag="wb")
        nc.vector.memset(wb[:], 0.0)

        pooled_all = sb.tile([128, BG * E], bf16, tag="pooled_all")
        with tc.tile_pool(name="psA", bufs=1, space="PSUM") as psA:
            acc = [
                psA.tile([128, 512], f32, tag=f"acc{g}", name=f"acc{g}")
                for g in range(BG)
            ]
            for w in range(NWARM):
                nc.tensor.matmul(out=acc[0][:, 0:E], lhsT=wa[:], rhs=wb[:],
                                 start=True, stop=True)
            for rep in range(repeat):
                for k in range(KC):
                    ct = cnt_t[k // 2]
                    t = k % 2
                    for g in range(BG):
                        nc.tensor.matmul(
                            out=acc[g][:, 0:E],
                            lhsT=ct[:, t * B + g * 128:t * B + (g + 1) * 128],
                            rhs=emb_t[k][:],
                            start=(k == 0),
                            stop=(k == KC - 1),
                        )
            # drain the accumulators (pipelines behind the last matmuls;
            # gpsimd cannot read PSUM)
            for g in range(BG):
                nc.vector.tensor_copy(
                    out=pooled_all[:, g * E:(g + 1) * E], in_=acc[g][:, 0:E]
                )

        # cross-core sum + scatter: core i keeps batch rows [128i, 128i+128)
        part_d = dram.tile([B, E], bf16)
        rs_d = dram.tile([BS, E], bf16)
        nc.gpsimd.dma_start(
            out=part_d[:].rearrange("(g p) e -> p g e", g=BG),
            in_=pooled_all[:].rearrange("p (g e) -> p g e", g=BG),
        )
        nc.gpsimd.collective_compute(
            "ReduceScatter",
            mybir.AluOpType.add,
            replica_groups=[list(range(NCORES))],
            ins=[part_d.opt()],
            outs=[rs_d.opt()],
        )

        with tc.tile_pool(name="ps", bufs=1, space="PSUM") as ps, \
                tc.tile_pool(name="ps2", bufs=2, space="PSUM") as ps2:
            # pooled^T [301, 128] (bias-ones row appended) as 3 e-chunks:
            # two 128-wide XBAR DMA-transposes straight from rs_d, and a
            # PE transpose for the 44-wide remainder
            ident = sb.tile([128, 128], bf16, tag="ident")
            make_identity(nc, ident[:])
            lhs = []
            for c in range(2):
                lt = sb.tile([128, 128], bf16, tag=f"lhs{c}", name=f"lhs{c}")
                eng = nc.sync if c == 0 else nc.scalar
                eng.dma_start_transpose(lt[:], rs_d[:, c * 128:(c + 1) * 128])
                lhs.append(lt)
            pooled2 = sb.tile([BS, 64], bf16, tag="pooled2")
            nc.gpsimd.dma_start(out=pooled2[:, 0:44], in_=rs_d[:, 256:300])
            pt = ps2.tile([64, 128], bf16, tag="tr", space="PSUM")
            nc.tensor.transpose(out=pt[0:44, :], in_=pooled2[:, 0:44],
                                identity=ident[:])
            lt2 = sb.tile([45, 128], bf16, tag="lhs2")
            nc.vector.memset(lt2[:], 1.0)
            nc.vector.tensor_copy(out=lt2[0:44, :], in_=pt[0:44, :])
            lhs.append(lt2)

            # fc1 flipped: hT[hc] = W1[:, hc]^T @ pooled^T (+ b1 via ones row)
            hT_ps = [
                ps.tile([128, 128], f32, tag=f"hT{hc}", name=f"hT{hc}")
                for hc in range(4)
            ]
            for hc in range(4):
                for c in range(3):
                    nc.tensor.matmul(
                        out=hT_ps[hc][:],
                        lhsT=w1_t[c][:, hc * 128:(hc + 1) * 128],
                        rhs=lhs[c][:],
                        start=(c == 0), stop=(c == 2),
                    )
            hT = sb.tile([128, 4 * 128], bf16, tag="hT")
            for hc in range(4):
                dst = hT[:, hc * 128:(hc + 1) * 128]
                if hc % 2 == 0:
                    nc.scalar.activation(
                        out=dst, in_=hT_ps[hc][:],
                        func=mybir.ActivationFunctionType.Relu)
                else:
                    nc.vector.tensor_scalar(
                        out=dst, in0=hT_ps[hc][:], scalar1=0.0, scalar2=None,
                        op0=mybir.AluOpType.max)

            # fc2: out = h @ W2 + b2 (hT is already the needed lhsT)
            ones1 = sb.tile([1, 128], bf16, tag="ones1")
            nc.vector.memset(ones1[:], 1.0)
            op_ = ps.tile([128, O], f32, tag="op", space="PSUM")
            for c in range(4):
                nc.tensor.matmul(
                    out=op_[:], lhsT=hT[:, c * 128:(c + 1) * 128],
                    rhs=w2_t[c][:], start=(c == 0), stop=False)
            nc.tensor.matmul(out=op_[:], lhsT=ones1[:], rhs=b2_t[:],
                             start=False, stop=True)
            out_sb = sb.tile([BS, O], f32, tag="osb")
            nc.vector.tensor_copy(out=out_sb[:], in_=op_[:])
            nc.gpsimd.dma_start(out=out_d[:], in_=out_sb[:])

    nc.finalize()
    return nc


def _prep_in_maps(text, lengths, emb_table, W1, b1, W2, b2):
    import ml_dtypes

    bf16 = ml_dtypes.bfloat16
    text = np.asarray(text, dtype=np.int64)         # [S, B]
    lengths = np.asarray(lengths, dtype=np.int64)   # [B]

    # counts^T [VP, B] scaled by 1/len: row v = per-batch frequency of
    # token v among the first len[b] positions (vocab-major for sharding)
    mask = np.arange(S)[:, None] < lengths[None, :]
    flat = (text * B + np.arange(B)[None, :])[mask]
    cntT = np.bincount(flat, minlength=VP * B).reshape(VP, B)
    inv_len = (1.0 / lengths.astype(np.float32)).astype(np.float32)
    cntT16 = (cntT * inv_len[None, :]).astype(bf16)

    embp = np.zeros((VP, E), np.float32)
    embp[:V] = np.asarray(emb_table, np.float32)
    emb16 = embp.astype(bf16)

    w1b = np.vstack([np.asarray(W1, np.float32),
                     np.asarray(b1, np.float32)[None, :]]).astype(bf16)
    w2b = np.vstack([np.asarray(W2, np.float32),
                     np.asarray(b2, np.float32)[None, :]]).astype(bf16)

    in_maps = []
    for i in range(NCORES):
        in_maps.append({
            "cnt": np.ascontiguousarray(cntT16[i * VSH:(i + 1) * VSH]),
            "emb": np.ascontiguousarray(emb16[i * VSH:(i + 1) * VSH]),
            "w1b": w1b,
            "w2b": w2b,
        })
    return in_maps


def _run(inputs, trace=False):
    from concourse.bass_utils import run_bass_kernel_spmd

    nc = _build_nc()
    in_maps = _prep_in_maps(**inputs)
    res = run_bass_kernel_spmd(nc, in_maps, list(range(NCORES)), trace=trace)
    out = np.concatenate([res.results[i]["out"] for i in range(NCORES)], axis=0)
    return out.astype(np.float32), res


def kernel(**inputs):
    out, _ = _run(inputs, trace=False)
    return out


# revision 23
# speedup vs baseline: 1.0957x; 1.0052x over previous
"""BOW classifier kernel for 8 Trainium2 NeuronCores.

Vocab-sharded counts-matmul formulation.  The masked mean-pool
  pooled[b] = (1/len[b]) * sum_{s<len[b]} emb[text[s,b]]
is a sparse matmul  pooled = counts @ emb  with counts[b,v] the number of
times token v appears in the first len[b] positions of column b (the
1/len is folded into counts on the host).  Each core owns a 6272-row
slice of the (padded, bf16) embedding table and the matching slice of
counts^T, computes its partial pooled on the tensor engine (bf16 x bf16
-> fp32 PSUM), and a bf16 ReduceScatter sums the partials and hands core
i batch rows [128*i, 128*(i+1)).  The MLP tail runs per-core on its 128
batch rows: pooled^T lands via XBAR DMA-transpose straight out of the
collective buffer, fc1 computes h^T = relu(W1^T pooled^T + b1) so fc2
(out = h @ W2 + b2) needs no transposes at all; bf16 inputs, fp32 PSUM.

Schedule notes: dummy matmuls on memset tiles ramp the PE pstate during
the initial DMA fill (the real accumulation opens with start=True, so
the junk is discarded); count/embedding DMAs interleave across the two
HWDGE queues (sync + scalar; counts two 128-row chunks per instruction)
while gpsimd carries only the small transfers, keeping every issue path
ahead of the PE's ~1.0 us/chunk consume rate.  The ReduceScatter
triggers as soon as the accumulator drains land; its start is pinned by
NRT's fixed first-collective barrier (~70 us), which the matmul phase
hides.
"""

import sys

import numpy as np

for _p in ("/opt/trn_rl_repo",):
    if _p not in sys.path:
        sys.path.insert(0, _p)

V, E, H, O = 50000, 300, 512, 2
S, B = 512, 1024
NCORES = 8
VSH = 6272          # padded vocab rows per core (49 * 128)
VP = NCORES * VSH   # 50176 padded vocab rows total
KC = VSH // 128     # 49 contraction chunks per core
BG = B // 128       # 8 batch groups of 128
BS = B // NCORES    # 128 batch rows per core after reduce-scatter
NWARM = 30          # dummy matmuls to ramp the PE pstate


def _build_nc(repeat=None, cnt_fp8=True):
    import os
    from contextlib import ExitStack

    if repeat is None:
        repeat = int(os.environ.get("KERNEL_REPEAT", "1"))

    import concourse.tile as tile
    from concourse import bacc, bass, mybir
    from concourse.masks import make_identity

    bf16, f32 = mybir.dt.bfloat16, mybir.dt.float32
    cdt = mybir.dt.float8e4 if cnt_fp8 else bf16

    nc = bacc.Bacc(None, target_bir_lowering=False, num_devices=NCORES)
    cnt_d = nc.declare_dram_parameter("cnt", [VSH, B], cdt, isOutput=False)
    emb_d = nc.declare_dram_parameter("emb", [VSH, E], bf16, isOutput=False)
    il_d = nc.declare_dram_parameter("ivl", [128, BG], f32, isOutput=False)
    w1b_d = nc.declare_dram_parameter("w1b", [E + 1, H], bf16, isOutput=False)
    w2b_d = nc.declare_dram_parameter("w2b", [H + 1, O], bf16, isOutput=False)
    out_d = nc.declare_dram_parameter("out", [BS, O], f32, isOutput=True)

    with tile.TileContext(nc) as tc, ExitStack() as ctx:
        sb = ctx.enter_context(tc.tile_pool(name="sb", bufs=1))
        dram = ctx.enter_context(tc.tile_pool(name="dram", bufs=1, space="DRAM"))

        # counts (two 128-row chunks per instruction) and embeddings,
        # interleaved in chunk order across the two HWDGE queues
        cnt_t, emb_t = [], []
        for j in range((KC + 1) // 2):
            r1 = min((j + 1) * 256, VSH)
            t2 = (r1 - j * 256) // 128
            ct = sb.tile([128, t2 * B], cdt, tag=f"cnt{j}", name=f"cnt{j}")
            eng_c = nc.sync if j % 2 == 0 else nc.scalar
            eng_e = nc.scalar if j % 2 == 0 else nc.sync
            eng_c.dma_start(
                out=ct[:].rearrange("p (t c) -> p t c", t=t2),
                in_=cnt_d[j * 256:r1, :].rearrange("(t p) c -> p t c", t=t2),
            )
            cnt_t.append(ct)
            for k in range(2 * j, 2 * j + t2):
                et = sb.tile([128, E], bf16, tag=f"emb{k}", name=f"emb{k}")
                eng_e.dma_start(out=et[:],
                                in_=emb_d[k * 128:(k + 1) * 128, :])
                emb_t.append(et)

        w1_t = []
        for c, (r0, r1) in enumerate([(0, 128), (128, 256), (256, E + 1)]):
            t = sb.tile([r1 - r0, H], bf16, tag=f"w1_{c}", name=f"w1_{c}")
            nc.gpsimd.dma_start(out=t[:], in_=w1b_d[r0:r1, :])
            w1_t.append(t)
        w2_t = []
        for c in range(4):
            t = sb.tile([128, O], bf16, tag=f"w2_{c}", name=f"w2_{c}")
            nc.gpsimd.dma_start(out=t[:], in_=w2b_d[c * 128:(c + 1) * 128, :])
            w2_t.append(t)
        b2_t = sb.tile([1, O], bf16, tag="b2")
        nc.gpsimd.dma_start(out=b2_t[:], in_=w2b_d[H:H + 1, :])
        ivl = sb.tile([128, BG], f32, tag="ivl")
        nc.gpsimd.dma_start(out=ivl[:], in_=il_d[:])

        # PE pstate warm-up on memset tiles (no DMA dependency); the real
        # accumulation below opens with start=True, discarding this junk
        wa = sb.tile([128, 128], bf16, tag="wa")
        nc.vector.memset(wa[:], 0.0)
        wb = sb.tile([128, E], bf16, tag="wb")
        nc.vector.memset(wb[:], 0.0)

        pooled_all = sb.tile([128, BG * E], bf16, tag="pooled_all")
        with tc.tile_pool(name="psA", bufs=1, space="PSUM") as psA:
            acc = [
                psA.tile([128, 512], f32, tag=f"acc{g}", name=f"acc{g}")
                for g in range(BG)
            ]
            for w in range(NWARM):
                nc.tensor.matmul(out=acc[0][:, 0:E], lhsT=wa[:], rhs=wb[:],
                                 start=True, stop=True)
            for rep in range(repeat):
                for k in range(KC):
                    ct = cnt_t[k // 2]
                    t = k % 2
                    for g in range(BG):
                        nc.tensor.matmul(
                            out=acc[g][:, 0:E],
                            lhsT=ct[:, t * B + g * 128:t * B + (g + 1) * 128],
                            rhs=emb_t[k][:],
                            start=(k == 0),
                            stop=(k == KC - 1),
                        )
            # drain the accumulators, folding in the 1/len scale (vector +
            # scalar in parallel; pipelines behind the last matmuls)
            for g in range(BG):
                dst = pooled_all[:, g * E:(g + 1) * E]
                if g % 2 == 0:
                    nc.vector.tensor_scalar(
                        out=dst, in0=acc[g][:, 0:E],
                        scalar1=ivl[:, g:g + 1], scalar2=None,
                        op0=mybir.AluOpType.mult,
                    )
                else:
                    nc.scalar.activation(
                        out=dst, in_=acc[g][:, 0:E],
                        func=mybir.ActivationFunctionType.Copy,
                        scale=ivl[:, g:g + 1],
                    )

        # cross-core sum + scatter: core i keeps batch rows [128i, 128i+128)
        # (bounce in two pieces so the trigger isn't gated on one big DMA)
        part_d = dram.tile([B, E], bf16)
        rs_d = dram.tile([BS, E], bf16)
        hB = BG // 2
        nc.sync.dma_start(
            out=part_d[0:hB * 128, :].rearrange("(g p) e -> p g e", g=hB),
            in_=pooled_all[:, 0:hB * E].rearrange("p (g e) -> p g e", g=hB),
        )
        nc.gpsimd.dma_start(
            out=part_d[hB * 128:, :].rearrange("(g p) e -> p g e", g=hB),
            in_=pooled_all[:, hB * E:].rearrange("p (g e) -> p g e", g=hB),
        )
        nc.gpsimd.collective_compute(
            "ReduceScatter",
            mybir.AluOpType.add,
            replica_groups=[list(range(NCORES))],
            ins=[part_d.opt()],
            outs=[rs_d.opt()],
        )

        with tc.tile_pool(name="ps", bufs=1, space="PSUM") as ps, \
                tc.tile_pool(name="ps2", bufs=2, space="PSUM") as ps2:
            # pooled^T [301, 128] (bias-ones row appended) as 3 e-chunks:
            # two 128-wide XBAR DMA-transposes straight from rs_d, and a
            # PE transpose for the 44-wide remainder
            ident = sb.tile([128, 128], bf16, tag="ident")
            make_identity(nc, ident[:])
            lhs = []
            for c in range(2):
                lt = sb.tile([128, 128], bf16, tag=f"lhs{c}", name=f"lhs{c}")
                eng = nc.sync if c == 0 else nc.scalar
                eng.dma_start_transpose(lt[:], rs_d[:, c * 128:(c + 1) * 128])
                lhs.append(lt)
            pooled2 = sb.tile([BS, 64], bf16, tag="pooled2")
            nc.gpsimd.dma_start(out=pooled2[:, 0:44], in_=rs_d[:, 256:300])
            pt = ps2.tile([64, 128], bf16, tag="tr", space="PSUM")
            nc.tensor.transpose(out=pt[0:44, :], in_=pooled2[:, 0:44],
                                identity=ident[:])
            lt2 = sb.tile([45, 128], bf16, tag="lhs2")
            nc.vector.memset(lt2[:], 1.0)
            nc.vector.tensor_copy(out=lt2[0:44, :], in_=pt[0:44, :])
            lhs.append(lt2)

            # fc1 flipped: hT[hc] = W1[:, hc]^T @ pooled^T (+ b1 via ones row)
            hT_ps = [
                ps.tile([128, 128], f32, tag=f"hT{hc}", name=f"hT{hc}")
                for hc in range(4)
            ]
            for hc in range(4):
                for c in range(3):
                    nc.tensor.matmul(
                        out=hT_ps[hc][:],
                        lhsT=w1_t[c][:, hc * 128:(hc + 1) * 128],
                        rhs=lhs[c][:],
                        start=(c == 0), stop=(c == 2),
                    )
            hT = sb.tile([128, 4 * 128], bf16, tag="hT")
            for hc in range(4):
                dst = hT[:, hc * 128:(hc + 1) * 128]
                if hc % 2 == 0:
                    nc.scalar.activation(
                        out=dst, in_=hT_ps[hc][:],
                        func=mybir.ActivationFunctionType.Relu)
                else:
                    nc.vector.tensor_scalar(
                        out=dst, in0=hT_ps[hc][:], scalar1=0.0, scalar2=None,
                        op0=mybir.AluOpType.max)

            # fc2: out = h @ W2 + b2 (hT is already the needed lhsT)
            ones1 = sb.tile([1, 128], bf16, tag="ones1")
            nc.vector.memset(ones1[:], 1.0)
            op_ = ps.tile([128, O], f32, tag="op", space="PSUM")
            for c in range(4):
                nc.tensor.matmul(
                    out=op_[:], lhsT=hT[:, c * 128:(c + 1) * 128],
                    rhs=w2_t[c][:], start=(c == 0), stop=False)
            nc.tensor.matmul(out=op_[:], lhsT=ones1[:], rhs=b2_t[:],
                             start=False, stop=True)
            out_sb = sb.tile([BS, O], f32, tag="osb")
            nc.vector.tensor_copy(out=out_sb[:], in_=op_[:])
            nc.gpsimd.dma_start(out=out_d[:], in_=out_sb[:])

    nc.finalize()
    return nc


def _prep_in_maps(text, lengths, emb_table, W1, b1, W2, b2):
    import ml_dtypes

    bf16 = ml_dtypes.bfloat16
    text = np.asarray(text, dtype=np.int64)         # [S, B]
    lengths = np.asarray(lengths, dtype=np.int64)   # [B]

    # counts^T [VP, B]: row v = per-batch count of token v among the
    # first len[b] positions (vocab-major for sharding); the 1/len scale
    # is applied on-device at accumulator-drain time
    mask = np.arange(S)[:, None] < lengths[None, :]
    flat = (text * B + np.arange(B)[None, :])[mask]
    cntT = np.bincount(flat, minlength=VP * B).reshape(VP, B)
    cnt_fp8 = cntT.max() <= 15  # integers <= 15 are exact in fp8e4m3
    cdt = ml_dtypes.float8_e4m3fn if cnt_fp8 else bf16
    cntT16 = cntT.astype(cdt)
    inv_len = (1.0 / lengths.astype(np.float32)).astype(np.float32)
    ivl = np.ascontiguousarray(inv_len.reshape(BG, 128).T)  # [128, BG]

    embp = np.zeros((VP, E), np.float32)
    embp[:V] = np.asarray(emb_table, np.float32)
    emb16 = embp.astype(bf16)

    w1b = np.vstack([np.asarray(W1, np.float32),
                     np.asarray(b1, np.float32)[None, :]]).astype(bf16)
    w2b = np.vstack([np.asarray(W2, np.float32),
                     np.asarray(b2, np.float32)[None, :]]).astype(bf16)

    in_maps = []
    for i in range(NCORES):
        in_maps.append({
            "cnt": np.ascontiguousarray(cntT16[i * VSH:(i + 1) * VSH]),
            "emb": np.ascontiguousarray(emb16[i * VSH:(i + 1) * VSH]),
            "ivl": ivl,
            "w1b": w1b,
            "w2b": w2b,
        })
    return in_maps, cnt_fp8


def _run(inputs, trace=False):
    from concourse.bass_utils import run_bass_kernel_spmd

    in_maps, cnt_fp8 = _prep_in_maps(**inputs)
    nc = _build_nc(cnt_fp8=cnt_fp8)
    res = run_bass_kernel_spmd(nc, in_maps, list(range(NCORES)), trace=trace)
    out = np.concatenate([res.results[i]["out"] for i in range(NCORES)], axis=0)
    return out.astype(np.float32), res


def kernel(**inputs):
    out, _ = _run(inputs, trace=False)
    return out
